# revision 8
# baseline (speedup 1.0000x reference)
"""Trainium2 Bass kernel for BilinearInteractionPlusLayer.

Math (per batch row b):
    pairs (i,j), i<j over F=40 fields, P=C(40,2)=780 pairs
    t[b,p,f] = sum_e x[b,i,e] * W[p,e,f]
    q[b,p]   = sum_f t[b,p,f] * x[b,j,f]
    h[b,d]   = sum_p q[b,p] * dense_w[p,d] + dense_b[d]
    out      = LayerNorm(h) * gamma + beta          (eps = 1e-3)

Sharding: data-parallel over batch, 2048 -> 256 rows on each of 8 cores.
W / dense_w / LN params are replicated. No collectives needed.

Per-core pipeline (all pair math in a transposed "[feature x batch]" layout):
  - x is cast to bf16 and transposed on-chip (DMA xbar) into the phase
    layout  xT[32*(j%4)+f, (j//4)*256 + b].
  - Stage 1 (PE): for field i and j-group jg (4 consecutive j), one matmul
      lhsT = Wcat chunk [32e x 128(c,f)]   (host-packed, zero-padded, strip i%4)
      rhs  = xT_i      [32e x 256b]
      out  = t chunk   [128(c,f) x 256b]  in PSUM (fp32)
  - DVE: m = t * xT-slice   (the batched bilinear "row dot" multiply)
  - PE:  h[16d x 256b] += dw2_chunk.T @ m_chunk  -- folds dense_w AND the
    reduction over (pair, f) into one accumulating matmul chain.
  - PE transpose of h, then LayerNorm (bn_stats/bn_aggr) per 128-row tile.
"""

import itertools

import numpy as np

import concourse.bass as bass
from concourse import bacc, masks, mybir
from concourse.bass_utils import run_bass_kernel_spmd
from concourse.tile import TileContext

F32 = mybir.dt.float32
BF16 = mybir.dt.bfloat16
NP_BF16 = mybir.dt.np(BF16)

B, F, E, P, D = 2048, 40, 32, 780, 16
NCORES = 8
BS = B // NCORES          # 256 batch rows per core
NJG = F // 4              # 10 j-groups of 4 fields
LN_EPS = 1e-3
SEG = 4                   # j-groups per PSUM segment (pipeline granule)


def _make_chunks():
    """One chunk = (field i, j-group jg): a [32e x 128(c,f)] stage-1 matmul."""
    chunks = []
    per_strip = [0, 0, 0, 0]
    for i in range(F):
        r = i % 4
        for jg in range((i + 1) // 4, NJG):
            chunks.append(
                {"i": i, "jg": jg, "r": r, "s": per_strip[r], "k": len(chunks)}
            )
            per_strip[r] += 1
    return chunks, per_strip


CHUNKS, PER_STRIP = _make_chunks()
CH_BY_IJG = {(c["i"], c["jg"]): c for c in CHUNKS}
NCH = len(CHUNKS)          # 210
SLOTS = max(PER_STRIP)     # 55 weight-chunk slots per partition strip


def _host_weights(W, dense_w):
    """Pack W into per-strip stationary chunks and dense_w into per-chunk
    [128(c,f) x 16d] reducers (broadcast over f, zero for absent pairs)."""
    pair_idx = {pq: n for n, pq in enumerate(itertools.combinations(range(F), 2))}
    wsb = np.zeros((128, SLOTS * 128), np.float32)
    dw2 = np.zeros((128, NCH * 16), np.float32)
    for ch in CHUNKS:
        i, jg, r, s, k = ch["i"], ch["jg"], ch["r"], ch["s"], ch["k"]
        for c in range(4):
            j = 4 * jg + c
            if j <= i:
                continue
            p = pair_idx[(i, j)]
            wsb[32 * r:32 * r + 32, s * 128 + 32 * c:s * 128 + 32 * c + 32] = W[p]
            dw2[32 * c:32 * c + 32, k * 16:(k + 1) * 16] = dense_w[p][None, :]
    return wsb.astype(NP_BF16), dw2.astype(NP_BF16)


def _host_xt(xc):
    """Per-core phase layout: xt[32*(j%4)+f, (j//4)*BS + b] = xc[b, j, f]."""
    arr = xc.transpose(1, 2, 0)                    # [F, E, BS]
    arr = arr.reshape(NJG, 4, E, BS)               # [jg, c, f, b]
    arr = arr.transpose(1, 2, 0, 3)                # [c, f, jg, b]
    return np.ascontiguousarray(arr.reshape(128, NJG * BS)).astype(NP_BF16)


def _build_bass():
    nc = bacc.Bacc(trn_type="TRN2")
    xin = nc.dram_tensor("xt", [128, NJG * BS], BF16, kind="ExternalInput")
    wsb = nc.dram_tensor("wsb", [128, SLOTS * 128], BF16, kind="ExternalInput")
    dw2 = nc.dram_tensor("dw2", [128, NCH * 16], BF16, kind="ExternalInput")
    vecs = nc.dram_tensor("vecs", [3, D], F32, kind="ExternalInput")
    out = nc.dram_tensor("out", [BS, D], F32, kind="ExternalOutput")

    with TileContext(nc) as tc:
        with (
            tc.tile_pool(name="const", bufs=1) as const,
            tc.tile_pool(name="mbuf", bufs=3) as mbuf,
            tc.tile_pool(name="tsegp", bufs=2, space="PSUM") as tsegp,
            tc.tile_pool(name="hpsp", bufs=1, space="PSUM") as hpsp,
            tc.tile_pool(name="lnp", bufs=2) as lnp,
        ):
            wsb_t = const.tile([128, SLOTS * 128], BF16)
            nc.sync.dma_start(out=wsb_t[:], in_=wsb[:, :])
            dw2_t = const.tile([128, NCH * 16], BF16)
            nc.sync.dma_start(out=dw2_t[:], in_=dw2[:, :])
            # vecs rows: 0 = dense_b, 1 = gamma, 2 = beta; broadcast across partitions
            vec_t = const.tile([128, 3, D], F32)
            src = vecs[:, :]
            nc.sync.dma_start(
                out=vec_t[:],
                in_=bass.AP(tensor=src.tensor, offset=src.offset,
                            ap=[[0, 128]] + [list(a) for a in src.ap]),
            )
            ident = const.tile([D, D], F32)
            masks.make_identity(nc, ident[:])
            eps_t = const.tile([128, 1], F32)
            nc.vector.memset(eps_t[:], LN_EPS)

            # ---- x arrives pre-transposed (host) in the phase layout
            xT = const.tile([128, NJG, BS], BF16)
            nc.sync.dma_start(out=xT[:], in_=xin[:, :])

            h_ps = hpsp.tile([D, BS], F32)

            # ---- main pair pipeline
            for i in range(F):
                r = i % 4
                jgs = list(range((i + 1) // 4, NJG))
                xrhs = xT[32 * r:32 * r + 32, i // 4, :]     # [32e x 256b]
                for s0 in range(0, len(jgs), SEG):
                    seg = jgs[s0:s0 + SEG]
                    t_ps = tsegp.tile([128, SEG, BS], F32, tag="t")
                    for u, jg in enumerate(seg):
                        ch = CH_BY_IJG[(i, jg)]
                        nc.tensor.matmul(
                            t_ps[:, u, :],
                            lhsT=wsb_t[32 * r:32 * r + 32,
                                       ch["s"] * 128:(ch["s"] + 1) * 128],
                            rhs=xrhs,
                            start=True, stop=True,
                            tile_position=(32 * r, 0),
                        )
                    m_t = mbuf.tile([128, SEG, BS], BF16, tag="m")
                    nc.vector.tensor_mul(
                        out=m_t[:, :len(seg), :],
                        in0=t_ps[:, :len(seg), :],
                        in1=xT[:, seg[0]:seg[0] + len(seg), :],
                    )
                    for u, jg in enumerate(seg):
                        k = CH_BY_IJG[(i, jg)]["k"]
                        nc.tensor.matmul(
                            h_ps[:, :],
                            lhsT=dw2_t[:, k * 16:(k + 1) * 16],
                            rhs=m_t[:, u, :],
                            start=(k == 0), stop=(k == NCH - 1),
                        )

            # ---- tail: transpose h, add bias, LayerNorm, write out
            h_sb = lnp.tile([D, BS], F32, tag="hsb")
            nc.scalar.copy(out=h_sb[:], in_=h_ps[:])
            ht_ps = hpsp.tile([128, BS // 128, D], F32)
            for bt in range(BS // 128):
                nc.tensor.transpose(
                    ht_ps[:, bt, :], h_sb[:, bt * 128:(bt + 1) * 128], ident[:]
                )
            for bt in range(BS // 128):
                hb = lnp.tile([128, D], F32, tag="hb")
                nc.vector.tensor_add(out=hb[:], in0=ht_ps[:, bt, :], in1=vec_t[:, 0, :])
                stats = lnp.tile([128, 6], F32, tag="stats")
                nc.vector.bn_stats(out=stats[:], in_=hb[:])
                mv = lnp.tile([128, 2], F32, tag="mv")
                nc.vector.bn_aggr(out=mv[:], in_=stats[:])
                nc.scalar.activation(
                    out=mv[:, 1:2], in_=mv[:, 1:2],
                    func=mybir.ActivationFunctionType.Sqrt,
                    bias=eps_t[:], scale=1.0,
                )
                nc.vector.reciprocal(out=mv[:, 1:2], in_=mv[:, 1:2])
                nc.vector.tensor_scalar(
                    out=hb[:], in0=hb[:],
                    scalar1=mv[:, 0:1], scalar2=mv[:, 1:2],
                    op0=mybir.AluOpType.subtract, op1=mybir.AluOpType.mult,
                )
                nc.vector.tensor_mul(out=hb[:], in0=hb[:], in1=vec_t[:, 1, :])
                nc.vector.tensor_add(out=hb[:], in0=hb[:], in1=vec_t[:, 2, :])
                nc.sync.dma_start(out=out[bt * 128:(bt + 1) * 128, :], in_=hb[:])
    nc.finalize()
    return nc


_NC_CACHE = None


def _get_nc():
    global _NC_CACHE
    if _NC_CACHE is None:
        _NC_CACHE = _build_bass()
    return _NC_CACHE


def run(x, W, dense_w, dense_b, gamma, beta, trace=False):
    x = np.asarray(x, np.float32)
    wsb_np, dw2_np = _host_weights(np.asarray(W, np.float32),
                                   np.asarray(dense_w, np.float32))
    vecs_np = np.stack([
        np.asarray(dense_b, np.float32),
        np.asarray(gamma, np.float32),
        np.asarray(beta, np.float32),
    ])
    in_maps = []
    for c in range(NCORES):
        in_maps.append({
            "xt": _host_xt(x[c * BS:(c + 1) * BS]),
            "wsb": wsb_np,
            "dw2": dw2_np,
            "vecs": vecs_np,
        })
    res = run_bass_kernel_spmd(
        _get_nc(), in_maps, core_ids=list(range(NCORES)), trace=trace
    )
    out = np.concatenate([res.results[c]["out"] for c in range(NCORES)], axis=0)
    return out.astype(np.float32), res


def kernel(x, W, dense_w, dense_b, gamma, beta):
    out, _ = run(x, W, dense_w, dense_b, gamma, beta)
    return out


# revision 15
# speedup vs baseline: 1.4532x; 1.4532x over previous
"""Trainium2 Bass kernel for BilinearInteractionPlusLayer.

Math (per batch row b):
    pairs (i,j), i<j over F=40 fields, P=C(40,2)=780 pairs
    t[b,p,f] = sum_e x[b,i,e] * W[p,e,f]
    q[b,p]   = sum_f t[b,p,f] * x[b,j,f]
    h[b,d]   = sum_p q[b,p] * dense_w[p,d] + dense_b[d]
    out      = LayerNorm(h) * gamma + beta          (eps = 1e-3)

Sharding: data-parallel over batch, 2048 -> 256 rows on each of 8 cores.
W / dense_w / LN params are replicated. No collectives.

Per-core pipeline (pair math in a transposed "[feature x batch]" layout):
  - x arrives pre-packed (host) as bf16 phase layout
        xT[32*(j%4)+f, (j//4)*256 + b]
  - Stage 1 (PE): per (field i, j-group jg) "chunk", one matmul
        lhsT = Wcat chunk [32e x 128(c,f)]  (host-packed, zeros for absent
               pairs, placed on partition strip i%4)
        rhs  = xT_i      [32e x 256b]
        out  = t chunk   [128(c,f) x 256b] in PSUM (fp32)
    Chunks of 4 consecutive i are interleaved so consecutive matmuls hit
    different row strips (tile_position row packing -> concurrent MMs,
    LDWEIGHTS hidden behind other strips' matmuls).
  - t is cast to bf16 on ScalarE (most segments), then VectorE multiplies
    by the matching xT slice at the 2x bf16 mode; every 4th segment skips
    the cast and multiplies fp32-from-PSUM directly on VectorE (1x) to
    balance the two engines.
  - PE folds dense_w AND the (pair,f) reduction into accumulating matmuls:
        h4[32g+d, b] += dw2_chunk.T @ m_chunk      (g = chunk % 4)
    col-tiled across 4 column groups -> 4 partial accumulators run
    concurrently in the PE array.
  - The 4 partial h's are transpose-accumulated (PE) into ht[b x d], then
    LayerNorm via bn_stats/bn_aggr per 128-row tile.
"""

import itertools

import numpy as np

import concourse.bass as bass
from concourse import bacc, mybir
from concourse.bass_utils import run_bass_kernel_spmd
from concourse.tile import TileContext

F32 = mybir.dt.float32
BF16 = mybir.dt.bfloat16
NP_BF16 = mybir.dt.np(BF16)

B, F, E, P, D = 2048, 40, 32, 780, 16
NCORES = 8
BS = B // NCORES          # 256 batch rows per core
NJG = F // 4              # 10 j-groups of 4 fields
LN_EPS = 1e-3
SEG = 4                   # j-group chunks per pipeline segment


def _make_chunks():
    """One chunk = (field i, j-group jg): a [32e x 128(c,f)] stage-1 matmul."""
    chunks = []
    per_strip = [0, 0, 0, 0]
    for i in range(F):
        r = i % 4
        for jg in range((i + 1) // 4, NJG):
            chunks.append(
                {"i": i, "jg": jg, "r": r, "s": per_strip[r], "k": len(chunks)}
            )
            per_strip[r] += 1
    return chunks, per_strip


CHUNKS, PER_STRIP = _make_chunks()
CH_BY_IJG = {(c["i"], c["jg"]): c for c in CHUNKS}
NCH = len(CHUNKS)          # 210
SLOTS = max(PER_STRIP)     # 55 weight-chunk slots per partition strip


def _segments_of(i):
    jgs = list(range((i + 1) // 4, NJG))
    return [jgs[s:s + SEG] for s in range(0, len(jgs), SEG)]


def _host_weights(W, dense_w):
    """Pack W into per-strip stationary chunks and dense_w into per-chunk
    [128(c,f) x 16d] reducers (broadcast over f, zero for absent pairs)."""
    pair_idx = {pq: n for n, pq in enumerate(itertools.combinations(range(F), 2))}
    wsb = np.zeros((128, SLOTS * 128), np.float32)
    dw2 = np.zeros((128, NCH * 16), np.float32)
    for ch in CHUNKS:
        i, jg, r, s, k = ch["i"], ch["jg"], ch["r"], ch["s"], ch["k"]
        for c in range(4):
            j = 4 * jg + c
            if j <= i:
                continue
            p = pair_idx[(i, j)]
            wsb[32 * r:32 * r + 32, s * 128 + 32 * c:s * 128 + 32 * c + 32] = W[p]
            dw2[32 * c:32 * c + 32, k * 16:(k + 1) * 16] = dense_w[p][None, :]
    return wsb.astype(NP_BF16), dw2.astype(NP_BF16)


def _host_xt(xc):
    """Per-core phase layout: xt[32*(j%4)+f, (j//4)*BS + b] = xc[b, j, f]."""
    arr = xc.transpose(1, 2, 0)                    # [F, E, BS]
    arr = arr.reshape(NJG, 4, E, BS)               # [jg, c, f, b]
    arr = arr.transpose(1, 2, 0, 3)                # [c, f, jg, b]
    return np.ascontiguousarray(arr.reshape(128, NJG * BS)).astype(NP_BF16)


def _host_ident4():
    """[128, 16] with a 16x16 identity at each 32-partition strip."""
    id4 = np.zeros((128, 16), np.float32)
    for g in range(4):
        id4[32 * g:32 * g + 16, :] = np.eye(16, dtype=np.float32)
    return id4


def _build_bass():
    nc = bacc.Bacc(trn_type="TRN2")
    xin = nc.dram_tensor("xt", [128, NJG * BS], BF16, kind="ExternalInput")
    wsb = nc.dram_tensor("wsb", [128, SLOTS * 128], BF16, kind="ExternalInput")
    dw2 = nc.dram_tensor("dw2", [128, NCH * 16], BF16, kind="ExternalInput")
    vecs = nc.dram_tensor("vecs", [3, D], F32, kind="ExternalInput")
    id4 = nc.dram_tensor("ident4", [128, D], F32, kind="ExternalInput")
    out = nc.dram_tensor("out", [BS, D], F32, kind="ExternalOutput")

    with TileContext(nc) as tc:
        with (
            tc.tile_pool(name="const", bufs=1) as const,
            tc.tile_pool(name="cast", bufs=3) as cbuf,
            tc.tile_pool(name="mbuf", bufs=3) as mbuf,
            tc.tile_pool(name="tsegp", bufs=3, space="PSUM") as tsegp,
            tc.tile_pool(name="hpsp", bufs=1, space="PSUM") as hpsp,
            tc.tile_pool(name="lnp", bufs=2) as lnp,
        ):
            # ---- constants / inputs
            xT = const.tile([128, NJG, BS], BF16)
            nc.sync.dma_start(out=xT[:], in_=xin[:, :])
            wsb_t = const.tile([128, SLOTS * 128], BF16)
            # split the weight DMA so the first matmuls aren't gated on the
            # whole 1.7 MB transfer
            wcut = 16 * 128
            nc.sync.dma_start(out=wsb_t[:, :wcut], in_=wsb[:, :wcut])
            nc.sync.dma_start(out=wsb_t[:, wcut:], in_=wsb[:, wcut:])
            dw2_t = const.tile([128, NCH * 16], BF16)
            dcut = 64 * 16
            nc.sync.dma_start(out=dw2_t[:, :dcut], in_=dw2[:, :dcut])
            nc.sync.dma_start(out=dw2_t[:, dcut:], in_=dw2[:, dcut:])
            # vecs rows: 0 = dense_b, 1 = gamma, 2 = beta; broadcast across
            # partitions
            vec_t = const.tile([128, 3, D], F32)
            src = vecs[:, :]
            nc.sync.dma_start(
                out=vec_t[:],
                in_=bass.AP(tensor=src.tensor, offset=src.offset,
                            ap=[[0, 128]] + [list(a) for a in src.ap]),
            )
            id4_t = const.tile([128, D], F32)
            nc.sync.dma_start(out=id4_t[:], in_=id4[:, :])
            eps_t = const.tile([128, 1], F32)
            nc.vector.memset(eps_t[:], LN_EPS)

            # h4: four col-tiled partial accumulators [16d x 256b] at
            # partition strips; hsum: their sum [16d x 256b]; ht: transposed
            # h [128b x (2 half, 16d)] -- packed into one PSUM tile.
            hcomb = hpsp.tile([128, 2 * BS + 2 * D], F32)
            h4 = hcomb[:, 0:BS]
            hsum = hcomb[0:D, BS:2 * BS]

            # ---- main pair pipeline: quads of fields across the 4 strips
            seg_counter = 0
            for gi in range(NJG):
                quad = [4 * gi + r for r in range(4)]
                segl = {i: _segments_of(i) for i in quad}
                nsteps = max(len(s) for s in segl.values())
                for step in range(nsteps):
                    active = [i for i in quad if step < len(segl[i])]
                    tps = {i: tsegp.tile([128, SEG, BS], F32, tag="t",
                                         name="tseg")
                           for i in active}
                    # stage-1 matmuls, chunk-interleaved across strips
                    maxc = max(len(segl[i][step]) for i in active)
                    for u in range(maxc):
                        for i in active:
                            seg = segl[i][step]
                            if u >= len(seg):
                                continue
                            ch = CH_BY_IJG[(i, seg[u])]
                            r = i % 4
                            nc.tensor.matmul(
                                tps[i][:, u, :],
                                lhsT=wsb_t[32 * r:32 * r + 32,
                                           ch["s"] * 128:(ch["s"] + 1) * 128],
                                rhs=xT[32 * r:32 * r + 32, i // 4, :],
                                start=True, stop=True,
                                tile_position=(32 * r, 0),
                            )
                    # elementwise multiply (+ dw2 fold-in matmuls)
                    for i in active:
                        seg = segl[i][step]
                        n = len(seg)
                        m_t = mbuf.tile([128, SEG, BS], BF16, tag="m")
                        if seg_counter % 4 == 3:
                            # direct fp32-from-PSUM multiply on VectorE
                            nc.vector.tensor_mul(
                                out=m_t[:, :n, :],
                                in0=tps[i][:, :n, :],
                                in1=xT[:, seg[0]:seg[0] + n, :],
                            )
                        else:
                            # ScalarE casts, VectorE multiplies at 2x bf16
                            tcast = cbuf.tile([128, SEG, BS], BF16, tag="tc")
                            nc.scalar.copy(out=tcast[:, :n, :],
                                           in_=tps[i][:, :n, :])
                            nc.vector.tensor_mul(
                                out=m_t[:, :n, :],
                                in0=tcast[:, :n, :],
                                in1=xT[:, seg[0]:seg[0] + n, :],
                            )
                        seg_counter += 1
                        for u, jg in enumerate(seg):
                            k = CH_BY_IJG[(i, jg)]["k"]
                            g = k % 4
                            nc.tensor.matmul(
                                h4[32 * g:32 * g + D, :],
                                lhsT=dw2_t[:, k * 16:(k + 1) * 16],
                                rhs=m_t[:, u, :],
                                start=(k < 4), stop=(k >= NCH - 4),
                                tile_position=(0, 32 * g),
                            )

            # ---- tail: combine the 4 partial h's with one selector matmul
            # (ident4 doubles as the selector: hsum[d,b] = sum_g h4[32g+d,b])
            hg_sb = lnp.tile([128, BS], F32, tag="hgsb")
            nc.vector.memset(hg_sb[:], 0.0)
            for g in range(4):
                nc.scalar.copy(out=hg_sb[32 * g:32 * g + D, :],
                               in_=h4[32 * g:32 * g + D, :])
            nc.tensor.matmul(hsum, lhsT=id4_t[:], rhs=hg_sb[:],
                             start=True, stop=True)
            hsum_sb = lnp.tile([D, BS], F32, tag="hsum_sb")
            nc.scalar.copy(out=hsum_sb[:], in_=hsum)

            def ht_v(half):
                off = 2 * BS + half * D
                return hcomb[0:128, off:off + D]

            for half in range(2):
                nc.tensor.transpose(
                    ht_v(half),
                    hsum_sb[:, half * 128:(half + 1) * 128],
                    id4_t[0:D, :],
                )
            # ---- LayerNorm per 128-row half
            for half in range(2):
                hb = lnp.tile([128, D], F32, tag="hb")
                nc.vector.tensor_add(out=hb[:], in0=ht_v(half),
                                     in1=vec_t[:, 0, :])
                stats = lnp.tile([128, 6], F32, tag="stats")
                nc.vector.bn_stats(out=stats[:], in_=hb[:])
                mv = lnp.tile([128, 2], F32, tag="mv")
                nc.vector.bn_aggr(out=mv[:], in_=stats[:])
                nc.scalar.activation(
                    out=mv[:, 1:2], in_=mv[:, 1:2],
                    func=mybir.ActivationFunctionType.Sqrt,
                    bias=eps_t[:], scale=1.0,
                )
                nc.vector.reciprocal(out=mv[:, 1:2], in_=mv[:, 1:2])
                nc.vector.tensor_scalar(
                    out=hb[:], in0=hb[:],
                    scalar1=mv[:, 0:1], scalar2=mv[:, 1:2],
                    op0=mybir.AluOpType.subtract, op1=mybir.AluOpType.mult,
                )
                nc.vector.tensor_mul(out=hb[:], in0=hb[:], in1=vec_t[:, 1, :])
                nc.vector.tensor_add(out=hb[:], in0=hb[:], in1=vec_t[:, 2, :])
                nc.sync.dma_start(out=out[half * 128:(half + 1) * 128, :],
                                  in_=hb[:])
    nc.finalize()
    return nc


_NC_CACHE = None


def _get_nc():
    global _NC_CACHE
    if _NC_CACHE is None:
        _NC_CACHE = _build_bass()
    return _NC_CACHE


def run(x, W, dense_w, dense_b, gamma, beta, trace=False):
    x = np.asarray(x, np.float32)
    wsb_np, dw2_np = _host_weights(np.asarray(W, np.float32),
                                   np.asarray(dense_w, np.float32))
    vecs_np = np.stack([
        np.asarray(dense_b, np.float32),
        np.asarray(gamma, np.float32),
        np.asarray(beta, np.float32),
    ])
    id4_np = _host_ident4()
    in_maps = []
    for c in range(NCORES):
        in_maps.append({
            "xt": _host_xt(x[c * BS:(c + 1) * BS]),
            "wsb": wsb_np,
            "dw2": dw2_np,
            "vecs": vecs_np,
            "ident4": id4_np,
        })
    res = run_bass_kernel_spmd(
        _get_nc(), in_maps, core_ids=list(range(NCORES)), trace=trace
    )
    out = np.concatenate([res.results[c]["out"] for c in range(NCORES)], axis=0)
    return out.astype(np.float32), res


def kernel(x, W, dense_w, dense_b, gamma, beta):
    out, _ = run(x, W, dense_w, dense_b, gamma, beta)
    return out


# revision 20
# speedup vs baseline: 1.4891x; 1.0247x over previous
"""Trainium2 Bass kernel for BilinearInteractionPlusLayer.

Math (per batch row b):
    pairs (i,j), i<j over F=40 fields, P=C(40,2)=780 pairs
    t[b,p,f] = sum_e x[b,i,e] * W[p,e,f]
    q[b,p]   = sum_f t[b,p,f] * x[b,j,f]
    h[b,d]   = sum_p q[b,p] * dense_w[p,d] + dense_b[d]
    out      = LayerNorm(h) * gamma + beta          (eps = 1e-3)

Sharding: data-parallel over batch, 2048 -> 256 rows on each of 8 cores.
W / dense_w / LN params are replicated. No collectives.

Per-core pipeline (pair math in a transposed "[feature x batch]" layout):
  - x arrives pre-packed (host) as bf16 phase layout
        xT[32*(j%4)+f, (j//4)*256 + b]
  - Stage 1 (PE): per (field i, j-group jg) "chunk", one matmul
        lhsT = Wcat chunk [32e x 128(c,f)]  (host-packed, zeros for absent
               pairs, placed on partition strip i%4)
        rhs  = xT_i      [32e x 256b]
        out  = t chunk   [128(c,f) x 256b] in PSUM (fp32)
    Chunks of 4 consecutive i are interleaved so consecutive matmuls hit
    different row strips (tile_position row packing -> concurrent MMs,
    LDWEIGHTS hidden behind other strips' matmuls).
  - t is cast to bf16 on ScalarE (most segments), then VectorE multiplies
    by the matching xT slice at the 2x bf16 mode; every 4th segment skips
    the cast and multiplies fp32-from-PSUM directly on VectorE (1x) to
    balance the two engines.
  - PE folds dense_w AND the (pair,f) reduction into accumulating matmuls:
        h4[32g+d, b] += dw2_chunk.T @ m_chunk      (g = chunk % 4)
    col-tiled across 4 column groups -> 4 partial accumulators run
    concurrently in the PE array.
  - The 4 partial h's are transpose-accumulated (PE) into ht[b x d], then
    LayerNorm via bn_stats/bn_aggr per 128-row tile.
"""

import itertools

import numpy as np

import concourse.bass as bass
from concourse import bacc, mybir
from concourse.bass_utils import run_bass_kernel_spmd
from concourse.tile import TileContext

F32 = mybir.dt.float32
BF16 = mybir.dt.bfloat16
NP_BF16 = mybir.dt.np(BF16)

B, F, E, P, D = 2048, 40, 32, 780, 16
NCORES = 8
BS = B // NCORES          # 256 batch rows per core
NJG = F // 4              # 10 j-groups of 4 fields
LN_EPS = 1e-3
SEG = 4                   # j-group chunks per pipeline segment


def _make_chunks():
    """One chunk = (field i, j-group jg): a [32e x 128(c,f)] stage-1 matmul."""
    chunks = []
    per_strip = [0, 0, 0, 0]
    for i in range(F):
        r = i % 4
        for jg in range((i + 1) // 4, NJG):
            chunks.append(
                {"i": i, "jg": jg, "r": r, "s": per_strip[r], "k": len(chunks)}
            )
            per_strip[r] += 1
    return chunks, per_strip


CHUNKS, PER_STRIP = _make_chunks()
CH_BY_IJG = {(c["i"], c["jg"]): c for c in CHUNKS}
NCH = len(CHUNKS)          # 210
SLOTS = max(PER_STRIP)     # 55 weight-chunk slots per partition strip


def _segments_of(i):
    jgs = list(range((i + 1) // 4, NJG))
    return [jgs[s:s + SEG] for s in range(0, len(jgs), SEG)]


def _host_weights(W, dense_w):
    """Pack W into per-strip stationary chunks and dense_w into per-chunk
    [128(c,f) x 16d] reducers (broadcast over f, zero for absent pairs)."""
    pair_idx = {pq: n for n, pq in enumerate(itertools.combinations(range(F), 2))}
    wsb = np.zeros((128, SLOTS * 128), np.float32)
    dw2 = np.zeros((128, NCH * 16), np.float32)
    for ch in CHUNKS:
        i, jg, r, s, k = ch["i"], ch["jg"], ch["r"], ch["s"], ch["k"]
        for c in range(4):
            j = 4 * jg + c
            if j <= i:
                continue
            p = pair_idx[(i, j)]
            wsb[32 * r:32 * r + 32, s * 128 + 32 * c:s * 128 + 32 * c + 32] = W[p]
            dw2[32 * c:32 * c + 32, k * 16:(k + 1) * 16] = dense_w[p][None, :]
    return wsb.astype(NP_BF16), dw2.astype(NP_BF16)


def _host_xt(xc):
    """Per-core phase layout: xt[32*(j%4)+f, (j//4)*BS + b] = xc[b, j, f]."""
    arr = xc.transpose(1, 2, 0)                    # [F, E, BS]
    arr = arr.reshape(NJG, 4, E, BS)               # [jg, c, f, b]
    arr = arr.transpose(1, 2, 0, 3)                # [c, f, jg, b]
    return np.ascontiguousarray(arr.reshape(128, NJG * BS)).astype(NP_BF16)


def _host_ident4():
    """[128, 16] with a 16x16 identity at each 32-partition strip."""
    id4 = np.zeros((128, 16), np.float32)
    for g in range(4):
        id4[32 * g:32 * g + 16, :] = np.eye(16, dtype=np.float32)
    return id4


def _build_bass():
    nc = bacc.Bacc(trn_type="TRN2")
    xin = nc.dram_tensor("xt", [128, NJG * BS], BF16, kind="ExternalInput")
    wsb = nc.dram_tensor("wsb", [128, SLOTS * 128], BF16, kind="ExternalInput")
    dw2 = nc.dram_tensor("dw2", [128, NCH * 16], BF16, kind="ExternalInput")
    vecs = nc.dram_tensor("vecs", [3, D], F32, kind="ExternalInput")
    id4 = nc.dram_tensor("ident4", [128, D], F32, kind="ExternalInput")
    out = nc.dram_tensor("out", [BS, D], F32, kind="ExternalOutput")

    with TileContext(nc) as tc:
        with (
            tc.tile_pool(name="const", bufs=1) as const,
            tc.tile_pool(name="cast", bufs=3) as cbuf,
            tc.tile_pool(name="mbuf", bufs=6) as mbuf,
            tc.tile_pool(name="tsegp", bufs=3, space="PSUM") as tsegp,
            tc.tile_pool(name="hpsp", bufs=1, space="PSUM") as hpsp,
            tc.tile_pool(name="lnp", bufs=2) as lnp,
        ):
            # ---- constants / inputs
            xT = const.tile([128, NJG, BS], BF16)
            nc.sync.dma_start(out=xT[:], in_=xin[:, :])
            wsb_t = const.tile([128, SLOTS * 128], BF16)
            # split the weight DMA so the first matmuls aren't gated on the
            # whole 1.7 MB transfer
            wcut = 16 * 128
            nc.sync.dma_start(out=wsb_t[:, :wcut], in_=wsb[:, :wcut])
            nc.sync.dma_start(out=wsb_t[:, wcut:], in_=wsb[:, wcut:])
            dw2_t = const.tile([128, NCH * 16], BF16)
            dcut = 64 * 16
            nc.sync.dma_start(out=dw2_t[:, :dcut], in_=dw2[:, :dcut])
            nc.sync.dma_start(out=dw2_t[:, dcut:], in_=dw2[:, dcut:])
            # vecs rows: 0 = dense_b, 1 = gamma, 2 = beta; broadcast across
            # partitions
            vec_t = const.tile([128, 3, D], F32)
            src = vecs[:, :]
            nc.sync.dma_start(
                out=vec_t[:],
                in_=bass.AP(tensor=src.tensor, offset=src.offset,
                            ap=[[0, 128]] + [list(a) for a in src.ap]),
            )
            id4_t = const.tile([128, D], F32)
            nc.sync.dma_start(out=id4_t[:], in_=id4[:, :])
            eps_t = const.tile([128, 1], F32)
            nc.vector.memset(eps_t[:], LN_EPS)

            # One PSUM bank holds: h4 (four col-tiled partial accumulators
            # [16d x 256b] at partition strips, free 0:256), hsum ([16d x
            # 256b], free 256:512), and ht reuses h4's range after h4 is
            # consumed.
            hcomb = hpsp.tile([128, 2 * BS], F32)
            h4 = hcomb[:, 0:BS]
            hsum = hcomb[0:D, BS:2 * BS]

            # ---- main pair pipeline: triples of consecutive fields (3
            # distinct strips, matching the 3 PSUM t-buffers), software
            # pipelined: a step's dw2 matmuls are emitted after the NEXT
            # step's stage-1 burst so the PE sees long same-kind runs.
            seg_counter = 0
            steps = []
            for t0 in range(0, F, 3):
                trip = [i for i in range(t0, min(t0 + 3, F))]
                segl = {i: _segments_of(i) for i in trip}
                nsteps = max(len(s) for s in segl.values())
                for step in range(nsteps):
                    steps.append([(i, segl[i][step]) for i in trip
                                  if step < len(segl[i])])
            # dw2 execution order (= flush order) to place start/stop flags
            ordered_k = [CH_BY_IJG[(i, jg)]["k"]
                         for active in steps for i, seg in active for jg in seg]
            first_k = {}
            last_k = {}
            for k in ordered_k:
                first_k.setdefault(k % 4, k)
                last_k[k % 4] = k

            pending = []          # list of (m_tile, [(u, k), ...])

            def flush_dw2():
                for m_t, ks in pending:
                    for u, k in ks:
                        g = k % 4
                        nc.tensor.matmul(
                            h4[32 * g:32 * g + D, :],
                            lhsT=dw2_t[:, k * 16:(k + 1) * 16],
                            rhs=m_t[:, u, :],
                            start=(first_k[g] == k),
                            stop=(last_k[g] == k),
                            tile_position=(0, 32 * g),
                        )
                pending.clear()
            for active in steps:
                tps = {i: tsegp.tile([128, SEG, BS], F32, tag="t",
                                     name="tseg")
                       for i, _ in active}
                # stage-1 matmul burst, chunk-interleaved across strips
                maxc = max(len(seg) for _, seg in active)
                for u in range(maxc):
                    for i, seg in active:
                        if u >= len(seg):
                            continue
                        ch = CH_BY_IJG[(i, seg[u])]
                        r = i % 4
                        nc.tensor.matmul(
                            tps[i][:, u, :],
                            lhsT=wsb_t[32 * r:32 * r + 32,
                                       ch["s"] * 128:(ch["s"] + 1) * 128],
                            rhs=xT[32 * r:32 * r + 32, i // 4, :],
                            start=True, stop=True,
                            tile_position=(32 * r, 0),
                        )
                # previous step's dw2 matmuls now (PE batch separation)
                flush_dw2()
                # elementwise multiply
                for i, seg in active:
                    n = len(seg)
                    m_t = mbuf.tile([128, SEG, BS], BF16, tag="m")
                    if seg_counter % 10 < 3:
                        # direct fp32-from-PSUM multiply on VectorE
                        nc.vector.tensor_mul(
                            out=m_t[:, :n, :],
                            in0=tps[i][:, :n, :],
                            in1=xT[:, seg[0]:seg[0] + n, :],
                        )
                    else:
                        # ScalarE casts, VectorE multiplies at 2x bf16
                        tcast = cbuf.tile([128, SEG, BS], BF16, tag="tc")
                        nc.scalar.copy(out=tcast[:, :n, :],
                                       in_=tps[i][:, :n, :])
                        nc.vector.tensor_mul(
                            out=m_t[:, :n, :],
                            in0=tcast[:, :n, :],
                            in1=xT[:, seg[0]:seg[0] + n, :],
                        )
                    seg_counter += 1
                    pending.append(
                        (m_t, [(u, CH_BY_IJG[(i, jg)]["k"])
                               for u, jg in enumerate(seg)]))
            flush_dw2()

            # ---- tail: combine the 4 partial h's with one selector matmul
            # (ident4 doubles as the selector: hsum[d,b] = sum_g h4[32g+d,b])
            hg_sb = lnp.tile([128, BS], F32, tag="hgsb")
            nc.vector.memset(hg_sb[:], 0.0)
            for g in range(4):
                nc.scalar.copy(out=hg_sb[32 * g:32 * g + D, :],
                               in_=h4[32 * g:32 * g + D, :])
            nc.tensor.matmul(hsum, lhsT=id4_t[:], rhs=hg_sb[:],
                             start=True, stop=True)
            hsum_sb = lnp.tile([D, BS], F32, tag="hsum_sb")
            nc.scalar.copy(out=hsum_sb[:], in_=hsum)

            def ht_v(half):
                # reuses h4's free range -- h4 is fully consumed by then
                off = half * D
                return hcomb[0:128, off:off + D]

            for half in range(2):
                nc.tensor.transpose(
                    ht_v(half),
                    hsum_sb[:, half * 128:(half + 1) * 128],
                    id4_t[0:D, :],
                )
            # ---- LayerNorm per 128-row half
            for half in range(2):
                hb = lnp.tile([128, D], F32, tag="hb")
                nc.vector.tensor_add(out=hb[:], in0=ht_v(half),
                                     in1=vec_t[:, 0, :])
                stats = lnp.tile([128, 6], F32, tag="stats")
                nc.vector.bn_stats(out=stats[:], in_=hb[:])
                mv = lnp.tile([128, 2], F32, tag="mv")
                nc.vector.bn_aggr(out=mv[:], in_=stats[:])
                nc.scalar.activation(
                    out=mv[:, 1:2], in_=mv[:, 1:2],
                    func=mybir.ActivationFunctionType.Sqrt,
                    bias=eps_t[:], scale=1.0,
                )
                nc.vector.reciprocal(out=mv[:, 1:2], in_=mv[:, 1:2])
                nc.vector.tensor_scalar(
                    out=hb[:], in0=hb[:],
                    scalar1=mv[:, 0:1], scalar2=mv[:, 1:2],
                    op0=mybir.AluOpType.subtract, op1=mybir.AluOpType.mult,
                )
                nc.vector.tensor_mul(out=hb[:], in0=hb[:], in1=vec_t[:, 1, :])
                nc.vector.tensor_add(out=hb[:], in0=hb[:], in1=vec_t[:, 2, :])
                nc.sync.dma_start(out=out[half * 128:(half + 1) * 128, :],
                                  in_=hb[:])
    nc.finalize()
    return nc


_NC_CACHE = None


def _get_nc():
    global _NC_CACHE
    if _NC_CACHE is None:
        _NC_CACHE = _build_bass()
    return _NC_CACHE


def run(x, W, dense_w, dense_b, gamma, beta, trace=False):
    x = np.asarray(x, np.float32)
    wsb_np, dw2_np = _host_weights(np.asarray(W, np.float32),
                                   np.asarray(dense_w, np.float32))
    vecs_np = np.stack([
        np.asarray(dense_b, np.float32),
        np.asarray(gamma, np.float32),
        np.asarray(beta, np.float32),
    ])
    id4_np = _host_ident4()
    in_maps = []
    for c in range(NCORES):
        in_maps.append({
            "xt": _host_xt(x[c * BS:(c + 1) * BS]),
            "wsb": wsb_np,
            "dw2": dw2_np,
            "vecs": vecs_np,
            "ident4": id4_np,
        })
    res = run_bass_kernel_spmd(
        _get_nc(), in_maps, core_ids=list(range(NCORES)), trace=trace
    )
    out = np.concatenate([res.results[c]["out"] for c in range(NCORES)], axis=0)
    return out.astype(np.float32), res


def kernel(x, W, dense_w, dense_b, gamma, beta):
    out, _ = run(x, W, dense_w, dense_b, gamma, beta)
    return out


# revision 23
# speedup vs baseline: 1.4944x; 1.0036x over previous
"""Trainium2 Bass kernel for BilinearInteractionPlusLayer.

Math (per batch row b):
    pairs (i,j), i<j over F=40 fields, P=C(40,2)=780 pairs
    t[b,p,f] = sum_e x[b,i,e] * W[p,e,f]
    q[b,p]   = sum_f t[b,p,f] * x[b,j,f]
    h[b,d]   = sum_p q[b,p] * dense_w[p,d] + dense_b[d]
    out      = LayerNorm(h) * gamma + beta          (eps = 1e-3)

Sharding: data-parallel over batch, 2048 -> 256 rows on each of 8 cores.
W / dense_w / LN params are replicated. No collectives.

Per-core pipeline (pair math in a transposed "[feature x batch]" layout):
  - x arrives pre-packed (host) as bf16 phase layout
        xT[32*(j%4)+f, (j//4)*256 + b]
  - Stage 1 (PE): per (field i, j-group jg) "chunk", one matmul
        lhsT = Wcat chunk [32e x 128(c,f)]  (host-packed, zeros for absent
               pairs, placed on partition strip i%4)
        rhs  = xT_i      [32e x 256b]
        out  = t chunk   [128(c,f) x 256b] in PSUM (fp32)
    Chunks of 4 consecutive i are interleaved so consecutive matmuls hit
    different row strips (tile_position row packing -> concurrent MMs,
    LDWEIGHTS hidden behind other strips' matmuls).
  - t is cast to bf16 on ScalarE (most segments), then VectorE multiplies
    by the matching xT slice at the 2x bf16 mode; every 4th segment skips
    the cast and multiplies fp32-from-PSUM directly on VectorE (1x) to
    balance the two engines.
  - PE folds dense_w AND the (pair,f) reduction into accumulating matmuls:
        h4[32g+d, b] += dw2_chunk.T @ m_chunk      (g = chunk % 4)
    col-tiled across 4 column groups -> 4 partial accumulators run
    concurrently in the PE array.
  - The 4 partial h's are transpose-accumulated (PE) into ht[b x d], then
    LayerNorm via bn_stats/bn_aggr per 128-row tile.
"""

import itertools

import numpy as np

import concourse.bass as bass
from concourse import bacc, mybir
from concourse.bass_utils import run_bass_kernel_spmd
from concourse.tile import TileContext

F32 = mybir.dt.float32
BF16 = mybir.dt.bfloat16
NP_BF16 = mybir.dt.np(BF16)

B, F, E, P, D = 2048, 40, 32, 780, 16
NCORES = 8
BS = B // NCORES          # 256 batch rows per core
NJG = F // 4              # 10 j-groups of 4 fields
LN_EPS = 1e-3
SEG = 4                   # j-group chunks per pipeline segment


def _make_chunks():
    """One chunk = (field i, j-group jg): a [32e x 128(c,f)] stage-1 matmul."""
    chunks = []
    per_strip = [0, 0, 0, 0]
    for i in range(F):
        r = i % 4
        for jg in range((i + 1) // 4, NJG):
            chunks.append(
                {"i": i, "jg": jg, "r": r, "s": per_strip[r], "k": len(chunks)}
            )
            per_strip[r] += 1
    return chunks, per_strip


CHUNKS, PER_STRIP = _make_chunks()
CH_BY_IJG = {(c["i"], c["jg"]): c for c in CHUNKS}
NCH = len(CHUNKS)          # 210
SLOTS = max(PER_STRIP)     # 55 weight-chunk slots per partition strip


def _segments_of(i):
    jgs = list(range((i + 1) // 4, NJG))
    return [jgs[s:s + SEG] for s in range(0, len(jgs), SEG)]


def _host_weights(W, dense_w):
    """Pack W into per-strip stationary chunks and dense_w into per-chunk
    [128(c,f) x 16d] reducers (broadcast over f, zero for absent pairs)."""
    pair_idx = {pq: n for n, pq in enumerate(itertools.combinations(range(F), 2))}
    wsb = np.zeros((128, SLOTS * 128), np.float32)
    dw2 = np.zeros((128, NCH * 16), np.float32)
    for ch in CHUNKS:
        i, jg, r, s, k = ch["i"], ch["jg"], ch["r"], ch["s"], ch["k"]
        for c in range(4):
            j = 4 * jg + c
            if j <= i:
                continue
            p = pair_idx[(i, j)]
            wsb[32 * r:32 * r + 32, s * 128 + 32 * c:s * 128 + 32 * c + 32] = W[p]
            dw2[32 * c:32 * c + 32, k * 16:(k + 1) * 16] = dense_w[p][None, :]
    return wsb.astype(NP_BF16), dw2.astype(NP_BF16)


def _host_xt(xc):
    """Per-core phase layout: xt[32*(j%4)+f, (j//4)*BS + b] = xc[b, j, f]."""
    arr = xc.transpose(1, 2, 0)                    # [F, E, BS]
    arr = arr.reshape(NJG, 4, E, BS)               # [jg, c, f, b]
    arr = arr.transpose(1, 2, 0, 3)                # [c, f, jg, b]
    return np.ascontiguousarray(arr.reshape(128, NJG * BS)).astype(NP_BF16)


def _host_ident4():
    """[128, 16] with a 16x16 identity at each 32-partition strip."""
    id4 = np.zeros((128, 16), np.float32)
    for g in range(4):
        id4[32 * g:32 * g + 16, :] = np.eye(16, dtype=np.float32)
    return id4


def _build_bass():
    nc = bacc.Bacc(trn_type="TRN2")
    xin = nc.dram_tensor("xt", [128, NJG * BS], BF16, kind="ExternalInput")
    wsb = nc.dram_tensor("wsb", [128, SLOTS * 128], BF16, kind="ExternalInput")
    dw2 = nc.dram_tensor("dw2", [128, NCH * 16], BF16, kind="ExternalInput")
    vecs = nc.dram_tensor("vecs", [3, D], F32, kind="ExternalInput")
    id4 = nc.dram_tensor("ident4", [128, D], F32, kind="ExternalInput")
    out = nc.dram_tensor("out", [BS, D], F32, kind="ExternalOutput")

    with TileContext(nc) as tc:
        with (
            tc.tile_pool(name="const", bufs=1) as const,
            tc.tile_pool(name="cast", bufs=6) as cbuf,
            tc.tile_pool(name="mbuf", bufs=6) as mbuf,
            tc.tile_pool(name="tsegp", bufs=3, space="PSUM") as tsegp,
            tc.tile_pool(name="hpsp", bufs=1, space="PSUM") as hpsp,
            tc.tile_pool(name="lnp", bufs=2) as lnp,
        ):
            # ---- constants / inputs
            xT = const.tile([128, NJG, BS], BF16)
            nc.sync.dma_start(out=xT[:], in_=xin[:, :])
            wsb_t = const.tile([128, SLOTS * 128], BF16)
            # split the weight DMAs so early matmuls aren't gated on the
            # whole transfer
            wcuts = [0, 8 * 128, 20 * 128, 36 * 128, SLOTS * 128]
            for a, b_ in zip(wcuts[:-1], wcuts[1:]):
                nc.sync.dma_start(out=wsb_t[:, a:b_], in_=wsb[:, a:b_])
            dw2_t = const.tile([128, NCH * 16], BF16)
            dcuts = [0, 32 * 16, 96 * 16, NCH * 16]
            for a, b_ in zip(dcuts[:-1], dcuts[1:]):
                nc.sync.dma_start(out=dw2_t[:, a:b_], in_=dw2[:, a:b_])
            # vecs rows: 0 = dense_b, 1 = gamma, 2 = beta; broadcast across
            # partitions
            vec_t = const.tile([128, 3, D], F32)
            src = vecs[:, :]
            nc.sync.dma_start(
                out=vec_t[:],
                in_=bass.AP(tensor=src.tensor, offset=src.offset,
                            ap=[[0, 128]] + [list(a) for a in src.ap]),
            )
            id4_t = const.tile([128, D], F32)
            nc.sync.dma_start(out=id4_t[:], in_=id4[:, :])
            eps_t = const.tile([128, 1], F32)
            nc.vector.memset(eps_t[:], LN_EPS)

            # One PSUM bank holds: h4 (four col-tiled partial accumulators
            # [16d x 256b] at partition strips, free 0:256), hsum ([16d x
            # 256b], free 256:512), and ht reuses h4's range after h4 is
            # consumed.
            hcomb = hpsp.tile([128, 2 * BS], F32)
            h4 = hcomb[:, 0:BS]
            hsum = hcomb[0:D, BS:2 * BS]

            # ---- main pair pipeline: triples of consecutive fields (3
            # distinct strips, matching the 3 PSUM t-buffers), software
            # pipelined: a step's dw2 matmuls are emitted after the NEXT
            # step's stage-1 burst so the PE sees long same-kind runs.
            seg_counter = 0
            steps = []
            for t0 in range(0, F, 4):
                quad = [i for i in range(t0, min(t0 + 4, F))]
                segl = {i: _segments_of(i) for i in quad}
                nsteps = max(len(s) for s in segl.values())
                for step in range(nsteps):
                    steps.append([(i, segl[i][step]) for i in quad
                                  if step < len(segl[i])])
            # dw2 execution order (= flush order) to place start/stop flags
            ordered_k = [CH_BY_IJG[(i, jg)]["k"]
                         for active in steps for i, seg in active for jg in seg]
            first_k = {}
            last_k = {}
            for k in ordered_k:
                first_k.setdefault(k % 4, k)
                last_k[k % 4] = k

            pending = []          # list of (m_tile, [(u, k), ...])

            def flush_dw2():
                for m_t, ks in pending:
                    for u, k in ks:
                        g = k % 4
                        nc.tensor.matmul(
                            h4[32 * g:32 * g + D, :],
                            lhsT=dw2_t[:, k * 16:(k + 1) * 16],
                            rhs=m_t[:, u, :],
                            start=(first_k[g] == k),
                            stop=(last_k[g] == k),
                            tile_position=(0, 32 * g),
                        )
                pending.clear()
            for active in steps:
                tps = {i: tsegp.tile([128, SEG, BS], F32, tag="t",
                                     name="tseg")
                       for i, _ in active}
                # stage-1 matmul burst, chunk-interleaved across strips
                maxc = max(len(seg) for _, seg in active)
                for u in range(maxc):
                    for i, seg in active:
                        if u >= len(seg):
                            continue
                        ch = CH_BY_IJG[(i, seg[u])]
                        r = i % 4
                        nc.tensor.matmul(
                            tps[i][:, u, :],
                            lhsT=wsb_t[32 * r:32 * r + 32,
                                       ch["s"] * 128:(ch["s"] + 1) * 128],
                            rhs=xT[32 * r:32 * r + 32, i // 4, :],
                            start=True, stop=True,
                            tile_position=(32 * r, 0),
                        )
                # previous step's dw2 matmuls now (PE batch separation)
                flush_dw2()
                # elementwise multiply
                for i, seg in active:
                    n = len(seg)
                    m_t = mbuf.tile([128, SEG, BS], BF16, tag="m")
                    if seg_counter % 10 < 3:
                        # direct fp32-from-PSUM multiply on VectorE
                        nc.vector.tensor_mul(
                            out=m_t[:, :n, :],
                            in0=tps[i][:, :n, :],
                            in1=xT[:, seg[0]:seg[0] + n, :],
                        )
                    else:
                        # ScalarE casts, VectorE multiplies at 2x bf16
                        tcast = cbuf.tile([128, SEG, BS], BF16, tag="tc")
                        nc.scalar.copy(out=tcast[:, :n, :],
                                       in_=tps[i][:, :n, :])
                        nc.vector.tensor_mul(
                            out=m_t[:, :n, :],
                            in0=tcast[:, :n, :],
                            in1=xT[:, seg[0]:seg[0] + n, :],
                        )
                    seg_counter += 1
                    pending.append(
                        (m_t, [(u, CH_BY_IJG[(i, jg)]["k"])
                               for u, jg in enumerate(seg)]))
            flush_dw2()

            # ---- tail: combine the 4 partial h's with one selector matmul
            # (ident4 doubles as the selector: hsum[d,b] = sum_g h4[32g+d,b])
            hg_sb = lnp.tile([128, BS], F32, tag="hgsb")
            nc.vector.memset(hg_sb[:], 0.0)
            for g in range(4):
                nc.scalar.copy(out=hg_sb[32 * g:32 * g + D, :],
                               in_=h4[32 * g:32 * g + D, :])
            nc.tensor.matmul(hsum, lhsT=id4_t[:], rhs=hg_sb[:],
                             start=True, stop=True)
            hsum_sb = lnp.tile([D, BS], F32, tag="hsum_sb")
            nc.scalar.copy(out=hsum_sb[:], in_=hsum)

            def ht_v(half):
                # reuses h4's free range -- h4 is fully consumed by then
                off = half * D
                return hcomb[0:128, off:off + D]

            for half in range(2):
                nc.tensor.transpose(
                    ht_v(half),
                    hsum_sb[:, half * 128:(half + 1) * 128],
                    id4_t[0:D, :],
                )
            # ---- LayerNorm per 128-row half
            for half in range(2):
                hb = lnp.tile([128, D], F32, tag="hb")
                nc.vector.tensor_add(out=hb[:], in0=ht_v(half),
                                     in1=vec_t[:, 0, :])
                stats = lnp.tile([128, 6], F32, tag="stats")
                nc.vector.bn_stats(out=stats[:], in_=hb[:])
                mv = lnp.tile([128, 2], F32, tag="mv")
                nc.vector.bn_aggr(out=mv[:], in_=stats[:])
                nc.scalar.activation(
                    out=mv[:, 1:2], in_=mv[:, 1:2],
                    func=mybir.ActivationFunctionType.Sqrt,
                    bias=eps_t[:], scale=1.0,
                )
                nc.vector.reciprocal(out=mv[:, 1:2], in_=mv[:, 1:2])
                nc.vector.tensor_scalar(
                    out=hb[:], in0=hb[:],
                    scalar1=mv[:, 0:1], scalar2=mv[:, 1:2],
                    op0=mybir.AluOpType.subtract, op1=mybir.AluOpType.mult,
                )
                nc.vector.tensor_mul(out=hb[:], in0=hb[:], in1=vec_t[:, 1, :])
                nc.vector.tensor_add(out=hb[:], in0=hb[:], in1=vec_t[:, 2, :])
                nc.sync.dma_start(out=out[half * 128:(half + 1) * 128, :],
                                  in_=hb[:])
    nc.finalize()
    return nc


_NC_CACHE = None


def _get_nc():
    global _NC_CACHE
    if _NC_CACHE is None:
        _NC_CACHE = _build_bass()
    return _NC_CACHE


def run(x, W, dense_w, dense_b, gamma, beta, trace=False):
    x = np.asarray(x, np.float32)
    wsb_np, dw2_np = _host_weights(np.asarray(W, np.float32),
                                   np.asarray(dense_w, np.float32))
    vecs_np = np.stack([
        np.asarray(dense_b, np.float32),
        np.asarray(gamma, np.float32),
        np.asarray(beta, np.float32),
    ])
    id4_np = _host_ident4()
    in_maps = []
    for c in range(NCORES):
        in_maps.append({
            "xt": _host_xt(x[c * BS:(c + 1) * BS]),
            "wsb": wsb_np,
            "dw2": dw2_np,
            "vecs": vecs_np,
            "ident4": id4_np,
        })
    res = run_bass_kernel_spmd(
        _get_nc(), in_maps, core_ids=list(range(NCORES)), trace=trace
    )
    out = np.concatenate([res.results[c]["out"] for c in range(NCORES)], axis=0)
    return out.astype(np.float32), res


def kernel(x, W, dense_w, dense_b, gamma, beta):
    out, _ = run(x, W, dense_w, dense_b, gamma, beta)
    return out


# revision 24
# speedup vs baseline: 1.5148x; 1.0136x over previous
"""Trainium2 Bass kernel for BilinearInteractionPlusLayer.

Math (per batch row b):
    pairs (i,j), i<j over F=40 fields, P=C(40,2)=780 pairs
    t[b,p,f] = sum_e x[b,i,e] * W[p,e,f]
    q[b,p]   = sum_f t[b,p,f] * x[b,j,f]
    h[b,d]   = sum_p q[b,p] * dense_w[p,d] + dense_b[d]
    out      = LayerNorm(h) * gamma + beta          (eps = 1e-3)

Sharding: data-parallel over batch, 2048 -> 256 rows on each of 8 cores.
W / dense_w / LN params are replicated. No collectives.

Per-core pipeline (pair math in a transposed "[feature x batch]" layout):
  - x arrives pre-packed (host) as bf16 phase layout
        xT[32*(j%4)+f, (j//4)*256 + b]
  - Stage 1 (PE): per (field i, j-group jg) "chunk", one matmul
        lhsT = Wcat chunk [32e x 128(c,f)]  (host-packed, zeros for absent
               pairs, placed on partition strip i%4)
        rhs  = xT_i      [32e x 256b]
        out  = t chunk   [128(c,f) x 256b] in PSUM (fp32)
    Chunks of 4 consecutive i are interleaved so consecutive matmuls hit
    different row strips (tile_position row packing -> concurrent MMs,
    LDWEIGHTS hidden behind other strips' matmuls).
  - t is cast to bf16 on ScalarE (most segments), then VectorE multiplies
    by the matching xT slice at the 2x bf16 mode; every 4th segment skips
    the cast and multiplies fp32-from-PSUM directly on VectorE (1x) to
    balance the two engines.
  - PE folds dense_w AND the (pair,f) reduction into accumulating matmuls:
        h4[32g+d, b] += dw2_chunk.T @ m_chunk      (g = chunk % 4)
    col-tiled across 4 column groups -> 4 partial accumulators run
    concurrently in the PE array.
  - The 4 partial h's are transpose-accumulated (PE) into ht[b x d], then
    LayerNorm via bn_stats/bn_aggr per 128-row tile.
"""

import itertools

import numpy as np

import concourse.bass as bass
from concourse import bacc, mybir
from concourse.bass_utils import run_bass_kernel_spmd
from concourse.tile import TileContext

F32 = mybir.dt.float32
BF16 = mybir.dt.bfloat16
NP_BF16 = mybir.dt.np(BF16)

B, F, E, P, D = 2048, 40, 32, 780, 16
NCORES = 8
BS = B // NCORES          # 256 batch rows per core
NJG = F // 4              # 10 j-groups of 4 fields
LN_EPS = 1e-3
SEG = 4                   # j-group chunks per pipeline segment


def _make_chunks():
    """One chunk = (field i, j-group jg): a [32e x 128(c,f)] stage-1 matmul."""
    chunks = []
    per_strip = [0, 0, 0, 0]
    for i in range(F):
        r = i % 4
        for jg in range((i + 1) // 4, NJG):
            chunks.append(
                {"i": i, "jg": jg, "r": r, "s": per_strip[r], "k": len(chunks)}
            )
            per_strip[r] += 1
    return chunks, per_strip


CHUNKS, PER_STRIP = _make_chunks()
CH_BY_IJG = {(c["i"], c["jg"]): c for c in CHUNKS}
NCH = len(CHUNKS)          # 210
SLOTS = max(PER_STRIP)     # 55 weight-chunk slots per partition strip


def _segments_of(i):
    jgs = list(range((i + 1) // 4, NJG))
    return [jgs[s:s + SEG] for s in range(0, len(jgs), SEG)]


def _host_weights(W, dense_w):
    """Pack W into per-strip stationary chunks and dense_w into per-chunk
    [128(c,f) x 16d] reducers (broadcast over f, zero for absent pairs)."""
    pair_idx = {pq: n for n, pq in enumerate(itertools.combinations(range(F), 2))}
    wsb = np.zeros((128, SLOTS * 128), np.float32)
    dw2 = np.zeros((128, NCH * 16), np.float32)
    for ch in CHUNKS:
        i, jg, r, s, k = ch["i"], ch["jg"], ch["r"], ch["s"], ch["k"]
        for c in range(4):
            j = 4 * jg + c
            if j <= i:
                continue
            p = pair_idx[(i, j)]
            wsb[32 * r:32 * r + 32, s * 128 + 32 * c:s * 128 + 32 * c + 32] = W[p]
            dw2[32 * c:32 * c + 32, k * 16:(k + 1) * 16] = dense_w[p][None, :]
    return wsb.astype(NP_BF16), dw2.astype(NP_BF16)


def _host_xt(xc):
    """Per-core phase layout: xt[32*(j%4)+f, (j//4)*BS + b] = xc[b, j, f]."""
    arr = xc.transpose(1, 2, 0)                    # [F, E, BS]
    arr = arr.reshape(NJG, 4, E, BS)               # [jg, c, f, b]
    arr = arr.transpose(1, 2, 0, 3)                # [c, f, jg, b]
    return np.ascontiguousarray(arr.reshape(128, NJG * BS)).astype(NP_BF16)


def _host_ident4():
    """[128, 16] with a 16x16 identity at each 32-partition strip."""
    id4 = np.zeros((128, 16), np.float32)
    for g in range(4):
        id4[32 * g:32 * g + 16, :] = np.eye(16, dtype=np.float32)
    return id4


def _build_bass():
    nc = bacc.Bacc(trn_type="TRN2")
    xin = nc.dram_tensor("xt", [128, NJG * BS], BF16, kind="ExternalInput")
    wsb = nc.dram_tensor("wsb", [128, SLOTS * 128], BF16, kind="ExternalInput")
    dw2 = nc.dram_tensor("dw2", [128, NCH * 16], BF16, kind="ExternalInput")
    vecs = nc.dram_tensor("vecs", [3, D], F32, kind="ExternalInput")
    id4 = nc.dram_tensor("ident4", [128, D], F32, kind="ExternalInput")
    out = nc.dram_tensor("out", [BS, D], F32, kind="ExternalOutput")

    with TileContext(nc) as tc:
        with (
            tc.tile_pool(name="const", bufs=1) as const,
            tc.tile_pool(name="cast", bufs=6) as cbuf,
            tc.tile_pool(name="mbuf", bufs=6) as mbuf,
            tc.tile_pool(name="tsegp", bufs=3, space="PSUM") as tsegp,
            tc.tile_pool(name="hpsp", bufs=1, space="PSUM") as hpsp,
            tc.tile_pool(name="lnp", bufs=2) as lnp,
        ):
            # ---- constants / inputs
            xT = const.tile([128, NJG, BS], BF16)
            nc.sync.dma_start(out=xT[:], in_=xin[:, :])
            wsb_t = const.tile([128, SLOTS * 128], BF16)
            # split the weight DMAs so early matmuls aren't gated on the
            # whole transfer
            wcuts = [0, 8 * 128, 20 * 128, 36 * 128, SLOTS * 128]
            for a, b_ in zip(wcuts[:-1], wcuts[1:]):
                nc.sync.dma_start(out=wsb_t[:, a:b_], in_=wsb[:, a:b_])
            dw2_t = const.tile([128, NCH * 16], BF16)
            dcuts = [0, 32 * 16, 96 * 16, NCH * 16]
            for a, b_ in zip(dcuts[:-1], dcuts[1:]):
                nc.sync.dma_start(out=dw2_t[:, a:b_], in_=dw2[:, a:b_])
            # vecs rows: 0 = dense_b, 1 = gamma, 2 = beta; broadcast across
            # partitions
            vec_t = const.tile([128, 3, D], F32)
            src = vecs[:, :]
            nc.sync.dma_start(
                out=vec_t[:],
                in_=bass.AP(tensor=src.tensor, offset=src.offset,
                            ap=[[0, 128]] + [list(a) for a in src.ap]),
            )
            id4_t = const.tile([128, D], F32)
            nc.sync.dma_start(out=id4_t[:], in_=id4[:, :])
            eps_t = const.tile([128, 1], F32)
            nc.vector.memset(eps_t[:], LN_EPS)

            # ---- PE clock warmup: dense full-array matmuls on scratch data
            # while the input DMAs land, to flip the HAM throttle to 8/8.
            scratch = const.tile([128, 512], BF16)
            nc.vector.memset(scratch[:], 0.25)
            wu_ps = tsegp.tile([128, SEG, BS], F32, tag="t", name="tseg")
            for w in range(18):
                nc.tensor.matmul(
                    wu_ps[:, 0:2, :].rearrange("p a b -> p (a b)"),
                    lhsT=scratch[:, 0:128], rhs=scratch[:],
                    start=True, stop=True,
                )

            # One PSUM bank holds: h4 (four col-tiled partial accumulators
            # [16d x 256b] at partition strips, free 0:256), hsum ([16d x
            # 256b], free 256:512), and ht reuses h4's range after h4 is
            # consumed.
            hcomb = hpsp.tile([128, 2 * BS], F32)
            h4 = hcomb[:, 0:BS]
            hsum = hcomb[0:D, BS:2 * BS]

            # ---- main pair pipeline: triples of consecutive fields (3
            # distinct strips, matching the 3 PSUM t-buffers), software
            # pipelined: a step's dw2 matmuls are emitted after the NEXT
            # step's stage-1 burst so the PE sees long same-kind runs.
            seg_counter = 0
            steps = []
            for t0 in range(0, F, 4):
                quad = [i for i in range(t0, min(t0 + 4, F))]
                segl = {i: _segments_of(i) for i in quad}
                nsteps = max(len(s) for s in segl.values())
                for step in range(nsteps):
                    steps.append([(i, segl[i][step]) for i in quad
                                  if step < len(segl[i])])
            # dw2 execution order (= flush order) to place start/stop flags
            ordered_k = [CH_BY_IJG[(i, jg)]["k"]
                         for active in steps for i, seg in active for jg in seg]
            first_k = {}
            last_k = {}
            for k in ordered_k:
                first_k.setdefault(k % 4, k)
                last_k[k % 4] = k

            pending = []          # list of (m_tile, [(u, k), ...])

            def flush_dw2():
                for m_t, ks in pending:
                    for u, k in ks:
                        g = k % 4
                        nc.tensor.matmul(
                            h4[32 * g:32 * g + D, :],
                            lhsT=dw2_t[:, k * 16:(k + 1) * 16],
                            rhs=m_t[:, u, :],
                            start=(first_k[g] == k),
                            stop=(last_k[g] == k),
                            tile_position=(0, 32 * g),
                        )
                pending.clear()
            for active in steps:
                tps = {i: tsegp.tile([128, SEG, BS], F32, tag="t",
                                     name="tseg")
                       for i, _ in active}
                # stage-1 matmul burst, chunk-interleaved across strips
                maxc = max(len(seg) for _, seg in active)
                for u in range(maxc):
                    for i, seg in active:
                        if u >= len(seg):
                            continue
                        ch = CH_BY_IJG[(i, seg[u])]
                        r = i % 4
                        nc.tensor.matmul(
                            tps[i][:, u, :],
                            lhsT=wsb_t[32 * r:32 * r + 32,
                                       ch["s"] * 128:(ch["s"] + 1) * 128],
                            rhs=xT[32 * r:32 * r + 32, i // 4, :],
                            start=True, stop=True,
                            tile_position=(32 * r, 0),
                        )
                # previous step's dw2 matmuls now (PE batch separation)
                flush_dw2()
                # elementwise multiply
                for i, seg in active:
                    n = len(seg)
                    m_t = mbuf.tile([128, SEG, BS], BF16, tag="m")
                    if seg_counter % 10 < 3:
                        # direct fp32-from-PSUM multiply on VectorE
                        nc.vector.tensor_mul(
                            out=m_t[:, :n, :],
                            in0=tps[i][:, :n, :],
                            in1=xT[:, seg[0]:seg[0] + n, :],
                        )
                    else:
                        # ScalarE casts, VectorE multiplies at 2x bf16
                        tcast = cbuf.tile([128, SEG, BS], BF16, tag="tc")
                        nc.scalar.copy(out=tcast[:, :n, :],
                                       in_=tps[i][:, :n, :])
                        nc.vector.tensor_mul(
                            out=m_t[:, :n, :],
                            in0=tcast[:, :n, :],
                            in1=xT[:, seg[0]:seg[0] + n, :],
                        )
                    seg_counter += 1
                    pending.append(
                        (m_t, [(u, CH_BY_IJG[(i, jg)]["k"])
                               for u, jg in enumerate(seg)]))
            flush_dw2()

            # ---- tail: combine the 4 partial h's with one selector matmul
            # (ident4 doubles as the selector: hsum[d,b] = sum_g h4[32g+d,b])
            hg_sb = lnp.tile([128, BS], F32, tag="hgsb")
            nc.vector.memset(hg_sb[:], 0.0)
            for g in range(4):
                nc.scalar.copy(out=hg_sb[32 * g:32 * g + D, :],
                               in_=h4[32 * g:32 * g + D, :])
            nc.tensor.matmul(hsum, lhsT=id4_t[:], rhs=hg_sb[:],
                             start=True, stop=True)
            hsum_sb = lnp.tile([D, BS], F32, tag="hsum_sb")
            nc.scalar.copy(out=hsum_sb[:], in_=hsum)

            def ht_v(half):
                # reuses h4's free range -- h4 is fully consumed by then
                off = half * D
                return hcomb[0:128, off:off + D]

            for half in range(2):
                nc.tensor.transpose(
                    ht_v(half),
                    hsum_sb[:, half * 128:(half + 1) * 128],
                    id4_t[0:D, :],
                )
            # ---- LayerNorm per 128-row half
            for half in range(2):
                hb = lnp.tile([128, D], F32, tag="hb")
                nc.vector.tensor_add(out=hb[:], in0=ht_v(half),
                                     in1=vec_t[:, 0, :])
                stats = lnp.tile([128, 6], F32, tag="stats")
                nc.vector.bn_stats(out=stats[:], in_=hb[:])
                mv = lnp.tile([128, 2], F32, tag="mv")
                nc.vector.bn_aggr(out=mv[:], in_=stats[:])
                nc.scalar.activation(
                    out=mv[:, 1:2], in_=mv[:, 1:2],
                    func=mybir.ActivationFunctionType.Sqrt,
                    bias=eps_t[:], scale=1.0,
                )
                nc.vector.reciprocal(out=mv[:, 1:2], in_=mv[:, 1:2])
                nc.vector.tensor_scalar(
                    out=hb[:], in0=hb[:],
                    scalar1=mv[:, 0:1], scalar2=mv[:, 1:2],
                    op0=mybir.AluOpType.subtract, op1=mybir.AluOpType.mult,
                )
                nc.vector.tensor_mul(out=hb[:], in0=hb[:], in1=vec_t[:, 1, :])
                nc.vector.tensor_add(out=hb[:], in0=hb[:], in1=vec_t[:, 2, :])
                nc.sync.dma_start(out=out[half * 128:(half + 1) * 128, :],
                                  in_=hb[:])
    nc.finalize()
    return nc


_NC_CACHE = None


def _get_nc():
    global _NC_CACHE
    if _NC_CACHE is None:
        _NC_CACHE = _build_bass()
    return _NC_CACHE


def run(x, W, dense_w, dense_b, gamma, beta, trace=False):
    x = np.asarray(x, np.float32)
    wsb_np, dw2_np = _host_weights(np.asarray(W, np.float32),
                                   np.asarray(dense_w, np.float32))
    vecs_np = np.stack([
        np.asarray(dense_b, np.float32),
        np.asarray(gamma, np.float32),
        np.asarray(beta, np.float32),
    ])
    id4_np = _host_ident4()
    in_maps = []
    for c in range(NCORES):
        in_maps.append({
            "xt": _host_xt(x[c * BS:(c + 1) * BS]),
            "wsb": wsb_np,
            "dw2": dw2_np,
            "vecs": vecs_np,
            "ident4": id4_np,
        })
    res = run_bass_kernel_spmd(
        _get_nc(), in_maps, core_ids=list(range(NCORES)), trace=trace
    )
    out = np.concatenate([res.results[c]["out"] for c in range(NCORES)], axis=0)
    return out.astype(np.float32), res


def kernel(x, W, dense_w, dense_b, gamma, beta):
    out, _ = run(x, W, dense_w, dense_b, gamma, beta)
    return out
